# revision 1
# baseline (speedup 1.0000x reference)
"""Conv2D 3x3 (stride 1, pad 1) Trainium2 Bass kernel.

Problem: x (32, 64, 64, 64) NCHW fp32, weight (128, 64, 3, 3) OIHW, bias (128,).
Output: (32, 128, 64, 64).

Strategy: data-parallel over batch across 8 cores (4 images/core). The host
pre-pads each image channel into a 66x66 zero-ringed layout (+ tail slack) and
pre-rounds x/weights to the PE's fp32r grid (1s+8e+11m, round-to-nearest).
On-chip, partitions 0-63 hold the padded channels and partitions 64-127 hold
the same data shifted down one padded row (a second DMA of the same HBM bytes
at offset 66), so a single K=128 matmul contracts two kernel-row taps at once.
Conv = 6 accumulating fp32r matmuls per 384-pixel PSUM tile (3 paired
ky={0,1} + 3 single ky=2); fp32r runs at full PE rate for moving dim >= 256.
Bias-add fuses into the PSUM->SBUF eviction on the scalar engine.
"""

import numpy as np

import concourse.bass as bass
import concourse.mybir as mybir
import concourse.tile as tile
from concourse import bacc
from concourse.bass_utils import run_bass_kernel_spmd
from concourse.tile_rust import add_dep_helper

N_CORES = 8
NIMG = 4  # images per core
C = 64  # input channels
H = W = 64
O = 128  # output channels
PW = 66  # padded row length
PH = 66  # padded rows
IMG = PH * PW  # 4356 padded elements per channel per image
QTOT = H * PW  # 4224 output positions in padded indexing (64 rows x 66)
# Row-aligned PSUM tiles: 10 groups of 6 output rows + 1 of 4 rows. Row
# alignment lets the eviction compact away the 2 garbage columns per row so
# the output staging buffer (and its store DMA) is fully contiguous.
TILE_ROWS = [6] * 10 + [4]
NQT = len(TILE_ROWS)  # 11
TAIL = 134  # slack so shifted reads stay in-bounds
XCOLS = IMG + TAIL  # 4490
UPLEN = QTOT + 8  # 4232: columns needed in the shifted upper half

F32 = mybir.dt.float32
F32R = mybir.dt.float32r

_CACHED_NC = None


def build_nc():
    nc = bacc.Bacc()
    x_in = nc.declare_dram_parameter("xp", [NIMG, C, XCOLS], F32R, isOutput=False)
    w_in = nc.declare_dram_parameter("wcat", [2 * C, 6, O], F32R, isOutput=False)
    b_in = nc.declare_dram_parameter("bias", [O, 1], F32, isOutput=False)
    out = nc.declare_dram_parameter("out", [NIMG, O, H, W], F32, isOutput=True)

    with tile.TileContext(nc) as tc:
        with (
            tc.tile_pool(name="const", bufs=1) as const_pool,
            tc.tile_pool(name="xp", bufs=4) as x_pool,
            tc.tile_pool(name="osb", bufs=2) as o_pool,
            tc.tile_pool(name="psum0", bufs=4, space="PSUM") as psum0_pool,
            tc.tile_pool(name="psum", bufs=4, space="PSUM") as psum_pool,
        ):
            wcat = const_pool.tile([2 * C, 6, O], F32R)
            bias_t = const_pool.tile([O, 1], F32)
            wcat_dma = nc.sync.dma_start(wcat[:, :, :], w_in[:, :, :])
            nc.sync.dma_start(bias_t[:, :], b_in[:, :])

            # Dummy 1x1 matmul reading only wcat: absorbs the weight-DMA
            # wait so the first real matmul carries a single sync wait (the
            # fused fp32r LDWEIGHTS+MM instruction has one wait slot).
            # (fp32r ISA: innermost free counts must be even, dst partition 0)
            warm = psum_pool.tile([2, 2], F32, tag="acc")
            warm_mm = nc.tensor.matmul(
                warm[:, :], wcat[0:1, 0, 0:2], wcat[0:1, 0, 0:2],
                start=True, stop=True,
            )

            for m in range(NIMG):
                xt = x_pool.tile([128, XCOLS], F32R)
                # lower half: padded image; upper half: same shifted one
                # padded row (pairs kernel rows ky=0/1 in one K=128 matmul).
                # Separate queues (SP HWDGE vs GPSIMD SWDGE) so the two loads
                # run concurrently and never queue behind output stores.
                nc.sync.dma_start(xt[0:C, :], x_in[m, :, :])
                nc.gpsimd.dma_start(
                    xt[C : 2 * C, 0:UPLEN], x_in[m, :, PW : PW + UPLEN]
                )

                osb = o_pool.tile([O, H * W], F32)
                r0 = 0
                for t in range(NQT):
                    rows = TILE_ROWS[t]
                    q0 = r0 * PW
                    qt = rows * PW
                    pool = psum0_pool if t == 0 else psum_pool
                    acc = pool.tile([O, 6 * PW], F32, tag="acc")
                    # ky=2 singles first: they read only the lower xt half,
                    # keeping per-matmul semaphore waits within the fused
                    # fp32r LDWEIGHTS+MM wait-slot budget.
                    for kx in range(3):
                        mm = nc.tensor.matmul(
                            acc[:, 0:qt],
                            wcat[0:C, 3 + kx, :],
                            xt[0:C, q0 + 2 * PW + kx : q0 + 2 * PW + kx + qt],
                            start=(kx == 0),
                            stop=False,
                        )
                        if m == 0 and t == 0 and kx == 0:
                            add_dep_helper(
                                mm.ins, warm_mm.ins, sync=False, reason="warm first"
                            )
                    for kx in range(3):
                        nc.tensor.matmul(
                            acc[:, 0:qt],
                            wcat[:, kx, :],
                            xt[0 : 2 * C, q0 + kx : q0 + kx + qt],
                            start=False,
                            stop=(kx == 2),
                        )
                    # evict + bias add on the scalar engine, dropping the 2
                    # garbage columns per row so osb is contiguous valid data
                    av = acc[:, 0:qt].rearrange("p (r c) -> p r c", c=PW)
                    ov = osb[:, r0 * W : (r0 + rows) * W].rearrange(
                        "p (r c) -> p r c", c=W
                    )
                    nc.scalar.activation(
                        ov[:, :, :],
                        av[:, :, 0:W],
                        mybir.ActivationFunctionType.Identity,
                        bias=bias_t[:, :],
                    )
                    r0 += rows

                # contiguous store on the ACT HWDGE queue
                nc.scalar.dma_start(out[m, :, :, :], osb[:, :])

    nc.compile()
    return nc


def _round_fp32r(a: np.ndarray) -> np.ndarray:
    """Round fp32 to the fp32r grid (11 mantissa bits, RNE)."""
    a = np.ascontiguousarray(a, dtype=np.float32)
    u = a.view(np.uint32)
    low = u & np.uint32(0xFFF)
    lsb = (u >> np.uint32(12)) & np.uint32(1)
    round_up = (low > 0x800) | ((low == 0x800) & (lsb == 1))
    r = (u & np.uint32(0xFFFFF000)) + (round_up.astype(np.uint32) << np.uint32(12))
    return r.view(np.float32)


def _prep_inputs(x, weight, bias):
    x = _round_fp32r(np.asarray(x, dtype=np.float32))
    n = x.shape[0]
    z = np.zeros((n, C, PH, PW), dtype=np.float32)
    z[:, :, 1 : 1 + H, 1 : 1 + W] = x
    xp = np.zeros((n, C, XCOLS), dtype=np.float32)
    xp[:, :, :IMG] = z.reshape(n, C, IMG)

    w_t = _round_fp32r(np.asarray(weight, dtype=np.float32)).transpose(1, 2, 3, 0)
    wcat = np.zeros((2 * C, 6, O), dtype=np.float32)
    wcat[0:C, 0:3, :] = w_t[:, 0, :, :]  # ky=0 (lower half of pairs)
    wcat[C : 2 * C, 0:3, :] = w_t[:, 1, :, :]  # ky=1 (upper half of pairs)
    wcat[0:C, 3:6, :] = w_t[:, 2, :, :]  # ky=2 singles
    b = np.ascontiguousarray(np.asarray(bias, dtype=np.float32).reshape(O, 1))
    return xp, wcat, b


def _in_maps(x, weight, bias):
    xp, wcat, b = _prep_inputs(x, weight, bias)
    return [
        {"xp": xp[i * NIMG : (i + 1) * NIMG], "wcat": wcat, "bias": b}
        for i in range(N_CORES)
    ]


def kernel(x: np.ndarray, weight: np.ndarray, bias: np.ndarray) -> np.ndarray:
    global _CACHED_NC
    if _CACHED_NC is None:
        _CACHED_NC = build_nc()
    res = run_bass_kernel_spmd(_CACHED_NC, _in_maps(x, weight, bias), list(range(N_CORES)))
    return np.concatenate([r["out"] for r in res.results], axis=0)


def run_profiled(x, weight, bias, tmpdir=None):
    """Dev helper: run with NTFF tracing, return BassKernelResults."""
    global _CACHED_NC
    if _CACHED_NC is None:
        _CACHED_NC = build_nc()
    return run_bass_kernel_spmd(
        _CACHED_NC, _in_maps(x, weight, bias), list(range(N_CORES)),
        trace=True, tmpdir=tmpdir,
    )



# revision 6
# speedup vs baseline: 2.0515x; 2.0515x over previous
"""Conv2D 3x3 (stride 1, pad 1) Trainium2 Bass kernel.

Problem: x (32, 64, 64, 64) NCHW fp32, weight (128, 64, 3, 3) OIHW, bias (128,).
Output: (32, 128, 64, 64).

Strategy: data-parallel over batch across 8 cores (4 images/core), bf16
matmuls (tolerance is 2e-2; bf16 with fp32 PSUM accumulation lands ~3e-3
and streams the PE at 1 col/cycle vs ~3 for fp32r). The host pre-pads each
image channel into a 66x66 zero-ringed layout (+ tail slack) in bf16.
On-chip, partitions 0-63 hold the padded channels and partitions 64-127
hold the same data shifted down one padded row (a second chunked DMA of
the same HBM bytes at offset 66), so a single K=128 matmul contracts two
kernel-row taps at once. Conv per 396-pixel PSUM tile = 3 paired K=128
matmuls (ky={0,1} x kx) + 3 K=64 matmuls for ky=2, two of which are
row-tiled to opposite PE halves so they stream concurrently -> 5 serial
streaming slots per tile. x DMAs are chunked so image-0 compute starts
after ~1/4 of the data. A dep-free warm-up matmul chain keeps the PE HAM
clock-gate at 2.4 GHz before real work lands. Bias-add fuses into the
PSUM->SBUF eviction on the scalar engine; output stores as bf16 and is
upcast on the host.
"""

import numpy as np
import ml_dtypes

import concourse.bass as bass
import concourse.mybir as mybir
import concourse.tile as tile
from concourse import bacc
from concourse.bass_utils import run_bass_kernel_spmd

N_CORES = 8
NIMG = 4  # images per core
C = 64  # input channels
H = W = 64
O = 128  # output channels
PW = 66  # padded row length
PH = 66  # padded rows
IMG = PH * PW  # 4356 padded elements per channel per image
# Row-aligned PSUM tiles: 10 groups of 6 output rows + 1 of 4 rows.
TILE_ROWS = [6] * 10 + [4]
NQT = len(TILE_ROWS)  # 11
# Lower copy: max read = 60*66 + 134 + 264 = 4358. Upper copy (shifted by
# one padded row): max read = 60*66 + 67 + 264 = 4291.
XCOLS = 4360
UPLEN = 4292
NCHUNK = 4  # x DMA chunks per copy per image
NWARM = 20  # dep-free warm-up matmuls to spin the HAM clock-gate up

import os
NWARM = int(os.environ.get("KWARM", NWARM))
ROWTILE = int(os.environ.get("KROWTILE", "1"))

F32 = mybir.dt.float32
BF16 = mybir.dt.bfloat16

_CACHED_NC = None


def build_nc():
    nc = bacc.Bacc()
    x_in = nc.declare_dram_parameter("xp", [NIMG, C, XCOLS], BF16, isOutput=False)
    w_in = nc.declare_dram_parameter("wcat", [2 * C, 6, O], BF16, isOutput=False)
    b_in = nc.declare_dram_parameter("bias", [O, 1], F32, isOutput=False)
    out = nc.declare_dram_parameter("out", [NIMG, O, H, W], BF16, isOutput=True)

    with tile.TileContext(nc) as tc:
        with (
            tc.tile_pool(name="const", bufs=1) as const_pool,
            tc.tile_pool(name="xp", bufs=3) as x_pool,
            tc.tile_pool(name="osb", bufs=2) as o_pool,
            tc.tile_pool(name="warm", bufs=1, space="PSUM") as warm_pool,
            tc.tile_pool(name="psum", bufs=6, space="PSUM") as psum_pool,
        ):
            # Warm-up chain: memset a scratch tile (no DMA dependency), then
            # issue back-to-back dummy matmuls. These run while the runtime
            # is still fetching instructions / loading the first x chunks,
            # so the HAM clock-gate is at K=8/8 (2.4 GHz) when real matmuls
            # start. They must finish before the first x chunk lands, so
            # they cost nothing if NWARM is sized right.
            if NWARM:
                scratch = const_pool.tile([128, 512], BF16)
                nc.gpsimd.memset(scratch[:, :], 0.0)
                warm = warm_pool.tile([128, 512], F32, tag="acc")
                for _ in range(NWARM):
                    nc.tensor.matmul(
                        warm[:, :], scratch[:, 0:128], scratch[:, :],
                        start=True, stop=True,
                    )

            wcat = const_pool.tile([2 * C, 6, O], BF16)
            bias_t = const_pool.tile([O, 1], F32)
            nc.sync.dma_start(wcat[:, :, :], w_in[:, :, :])
            nc.sync.dma_start(bias_t[:, :], b_in[:, :])

            lb = [XCOLS * k // NCHUNK for k in range(NCHUNK + 1)]
            ub = [UPLEN * k // NCHUNK for k in range(NCHUNK + 1)]
            for m in range(NIMG):
                xt = x_pool.tile([128, XCOLS], BF16)
                # lower half: padded image; upper half: same shifted one
                # padded row (pairs kernel rows ky=0/1 in one K=128 matmul).
                # Chunked + interleaved so tile 0's matmuls only wait for
                # the first ~1/NCHUNK of the image.
                for k in range(NCHUNK):
                    nc.sync.dma_start(
                        xt[0:C, lb[k] : lb[k + 1]], x_in[m, :, lb[k] : lb[k + 1]]
                    )
                    nc.sync.dma_start(
                        xt[C : 2 * C, ub[k] : ub[k + 1]],
                        x_in[m, :, PW + ub[k] : PW + ub[k + 1]],
                    )

                osb = o_pool.tile([O, H * W], BF16)
                r0 = 0
                for t in range(NQT):
                    rows = TILE_ROWS[t]
                    q0 = r0 * PW
                    qt = rows * PW
                    acc = psum_pool.tile([O, 6 * PW], F32, tag="acc")
                    # 3 paired K=128 matmuls: ky=0 (lower half) + ky=1
                    # (upper half) for each kx.
                    for kx in range(3):
                        nc.tensor.matmul(
                            acc[:, 0:qt],
                            wcat[:, kx, :],
                            xt[0 : 2 * C, q0 + kx : q0 + kx + qt],
                            start=(kx == 0),
                            stop=False,
                        )
                    # ky=2 taps: kx=0 on PE rows 0-63 and kx=1 on rows
                    # 64-127 (reading the shifted copy at offset 67) are
                    # row-tiled so they stream concurrently; kx=2 closes
                    # the accumulation group.
                    nc.tensor.matmul(
                        acc[:, 0:qt],
                        wcat[0:C, 3, :],
                        xt[0:C, q0 + 132 : q0 + 132 + qt],
                        start=False,
                        stop=False,
                    )
                    if ROWTILE:
                        nc.tensor.matmul(
                            acc[:, 0:qt],
                            wcat[C : 2 * C, 3, :],
                            xt[C : 2 * C, q0 + 67 : q0 + 67 + qt],
                            start=False,
                            stop=False,
                        )
                    else:
                        nc.tensor.matmul(
                            acc[:, 0:qt],
                            wcat[0:C, 5, :],
                            xt[0:C, q0 + 133 : q0 + 133 + qt],
                            start=False,
                            stop=False,
                        )
                    nc.tensor.matmul(
                        acc[:, 0:qt],
                        wcat[0:C, 4, :],
                        xt[0:C, q0 + 134 : q0 + 134 + qt],
                        start=False,
                        stop=True,
                    )
                    # evict + bias add on the scalar engine, dropping the 2
                    # garbage columns per row so osb is contiguous valid data
                    av = acc[:, 0:qt].rearrange("p (r c) -> p r c", c=PW)
                    ov = osb[:, r0 * W : (r0 + rows) * W].rearrange(
                        "p (r c) -> p r c", c=W
                    )
                    nc.scalar.activation(
                        ov[:, :, :],
                        av[:, :, 0:W],
                        mybir.ActivationFunctionType.Identity,
                        bias=bias_t[:, :],
                    )
                    r0 += rows

                # contiguous store on the ACT HWDGE queue
                nc.scalar.dma_start(out[m, :, :, :], osb[:, :])

    nc.compile()
    return nc


def _prep_inputs(x, weight, bias):
    bf16 = ml_dtypes.bfloat16
    x = np.asarray(x, dtype=np.float32)
    n = x.shape[0]
    z = np.zeros((n, C, PH, PW), dtype=bf16)
    z[:, :, 1 : 1 + H, 1 : 1 + W] = x.astype(bf16)
    xp = np.zeros((n, C, XCOLS), dtype=bf16)
    xp[:, :, :IMG] = z.reshape(n, C, IMG)

    w_t = np.asarray(weight, dtype=np.float32).astype(bf16).transpose(1, 2, 3, 0)
    wcat = np.zeros((2 * C, 6, O), dtype=bf16)
    wcat[0:C, 0:3, :] = w_t[:, 0, :, :]  # ky=0 (lower half of pairs)
    wcat[C : 2 * C, 0:3, :] = w_t[:, 1, :, :]  # ky=1 (upper half of pairs)
    wcat[0:C, 3, :] = w_t[:, 2, 0, :]  # ky=2 kx=0 (lower rows)
    wcat[C : 2 * C, 3, :] = w_t[:, 2, 1, :]  # ky=2 kx=1 (upper rows)
    wcat[0:C, 4, :] = w_t[:, 2, 2, :]  # ky=2 kx=2 (lower rows)
    wcat[0:C, 5, :] = w_t[:, 2, 1, :]  # ky=2 kx=1 (lower rows, non-rowtile fallback)
    b = np.ascontiguousarray(np.asarray(bias, dtype=np.float32).reshape(O, 1))
    return xp, wcat, b


def _in_maps(x, weight, bias):
    xp, wcat, b = _prep_inputs(x, weight, bias)
    return [
        {"xp": xp[i * NIMG : (i + 1) * NIMG], "wcat": wcat, "bias": b}
        for i in range(N_CORES)
    ]


def kernel(x: np.ndarray, weight: np.ndarray, bias: np.ndarray) -> np.ndarray:
    global _CACHED_NC
    if _CACHED_NC is None:
        _CACHED_NC = build_nc()
    res = run_bass_kernel_spmd(_CACHED_NC, _in_maps(x, weight, bias), list(range(N_CORES)))
    return np.concatenate([r["out"] for r in res.results], axis=0).astype(np.float32)


def run_profiled(x, weight, bias, tmpdir=None):
    """Dev helper: run with NTFF tracing, return BassKernelResults."""
    global _CACHED_NC
    if _CACHED_NC is None:
        _CACHED_NC = build_nc()
    return run_bass_kernel_spmd(
        _CACHED_NC, _in_maps(x, weight, bias), list(range(N_CORES)),
        trace=True, tmpdir=tmpdir,
    )
